# revision 1
# baseline (speedup 1.0000x reference)
"""Trainium2 Bass kernel for causal multi-head attention (B=4, T=2048, D=1024, H=16).

Sharding: 8 cores = 4 batches x 2 head-groups (8 heads each).
Per core pipeline (Tile framework, single SPMD program):
  phase 1: Q/K projections into transposed per-head-pair layout QT/KT [128=2*64, T],
           V projection into [t, 8*65] layout (65th col per head = ones, for rowsums)
  phase 2: per (q-range of 512, head-pair): causal flash attention in transposed
           layout: ST[k,q] = KT-slice^T @ QT-slice (row-packed pair of fp32r matmuls),
           PT = exp(ST) (ACT), causal mask on diagonal tiles (DVE mul),
           OT[hd+1, q] += [V|1]^T @ PT (bf16), normalize by reciprocal rowsum.
  phase 3: output projection YT[dout, t] = Wo_gT^T @ OT (bf16) + bias (g=0 adds bo)
  ReduceScatter(add) across the batch pair -> each core outputs its dout half.
Host: transpose/slice weights, assemble [B, T, D] from per-core [512, T] halves.
"""

import numpy as np

B, T, D, H, HD = 4, 2048, 1024, 16, 64
NCORES = 8
NP = 4          # head pairs per core
NJ = 4          # q-ranges of 512
QW = 512
TB = T // 128   # 16

_CACHE = {}


def _build_nc():
    import concourse.mybir as mybir
    import concourse.tile as tile
    from concourse import bacc

    F32 = mybir.dt.float32
    F32R = mybir.dt.float32r
    BF16 = mybir.dt.bfloat16
    F16 = mybir.dt.float16
    AF = mybir.ActivationFunctionType
    ALU = mybir.AluOpType

    nc = bacc.Bacc(None, target_bir_lowering=False)
    xt_d = nc.declare_dram_parameter("xt", [D, T], BF16, isOutput=False)
    wq_d = nc.declare_dram_parameter("wq", [D, 512], BF16, isOutput=False)
    wk_d = nc.declare_dram_parameter("wk", [D, 512], BF16, isOutput=False)
    wv_d = nc.declare_dram_parameter("wv", [D, 512], BF16, isOutput=False)
    wo_d = nc.declare_dram_parameter("wo", [512, D], BF16, isOutput=False)
    bias_d = nc.declare_dram_parameter("bias", [128, 8], F32, isOutput=False)
    mask_d = nc.declare_dram_parameter("mask", [128, 512], BF16, isOutput=False)
    yt_d = nc.declare_dram_parameter("yt", [512, T], F32, isOutput=True)

    RG = [[0, 1], [2, 3], [4, 5], [6, 7]]

    with tile.TileContext(nc) as tc:
        with (
            tc.tile_pool(name="persist", bufs=1) as pers,
            tc.tile_pool(name="work", bufs=1) as work,
            tc.tile_pool(name="dram", bufs=1, space="DRAM") as dram,
            tc.tile_pool(name="psum", bufs=1, space="PSUM") as psum,
        ):
            qt = pers.tile([128, NP, T], BF16)
            kt = pers.tile([128, NP, T], BF16)
            v = pers.tile([128, TB, 8 * 65], BF16)
            ot = pers.tile([128, NP, T], BF16)
            m0 = pers.tile([128, 512], BF16)
            wo = pers.tile([128, 4, D], BF16)
            bias = pers.tile([128, 8], F32)
            wq = pers.tile([128, 8, 512], BF16)
            wk = pers.tile([128, 8, 512], BF16)
            wv = pers.tile([128, 8, 512], BF16)
            nc.sync.dma_start(out=m0[:], in_=mask_d[:])
            nc.sync.dma_start(out=bias[:], in_=bias_d[:])
            nc.sync.dma_start(out=wo[:], in_=wo_d.rearrange("(c p) n -> p c n", p=128))
            for c in range(8):
                nc.sync.dma_start(out=wq[:, c, :], in_=wq_d[c * 128:(c + 1) * 128, :])
                nc.sync.dma_start(out=wk[:, c, :], in_=wk_d[c * 128:(c + 1) * 128, :])
                nc.sync.dma_start(out=wv[:, c, :], in_=wv_d[c * 128:(c + 1) * 128, :])

            yt_part = dram.tile([NJ, 1024, QW], F32)
            yt_rs = dram.tile([NJ, 512, QW], F32)

            for j in range(NJ):
                # ---------------- phase 1(j): projections for this t-range ----------------
                xsl = []
                for c in range(8):
                    xs = work.tile([128, QW], BF16, tag="xs", bufs=12)
                    nc.sync.dma_start(
                        out=xs[:], in_=xt_d[c * 128:(c + 1) * 128, j * QW:(j + 1) * QW]
                    )
                    xsl.append(xs)
                for p in range(NP):
                    for w_sb, dst in ((wq, qt), (wk, kt)):
                        acc = psum.tile([128, QW], F32, tag="small", bufs=2)
                        for c in range(8):
                            nc.tensor.matmul(
                                acc[:],
                                w_sb[:, c, p * 128:(p + 1) * 128],
                                xsl[c][:],
                                start=(c == 0),
                                stop=(c == 7),
                            )
                        nc.vector.tensor_copy(dst[:, p, j * QW:(j + 1) * QW], acc[:])
                for sub in range(4):
                    i = 4 * j + sub
                    acc = psum.tile([128, QW], F32, tag="small", bufs=2)
                    for c in range(8):
                        nc.tensor.matmul(
                            acc[:],
                            xsl[c][:, sub * 128:(sub + 1) * 128],
                            wv[:, c, :],
                            start=(c == 0),
                            stop=(c == 7),
                        )
                    vblk = v[:, i, :].rearrange("p (h c) -> p h c", c=65)
                    nc.vector.tensor_copy(
                        vblk[:, :, 0:64],
                        acc[:].rearrange("p (h c) -> p h c", c=64),
                    )
                    nc.gpsimd.memset(vblk[:, :, 64:65], 1.0)

                # ---------------- phase 2(j): attention ----------------
                for p in range(NP):
                    hA, hB = 2 * p, 2 * p + 1
                    o_A = psum.tile([65, QW], F32, tag="o", bufs=2)
                    o_B = psum.tile([65, QW], F32, tag="o", bufs=2)
                    nkb = 4 * j + 4
                    for kb in range(nkb):
                        o = kb - 4 * j  # diagonal offset; < 0 means full block
                        lo = 128 * o if o > 0 else 0  # first live q col in range
                        W = QW - lo
                        st = psum.tile([128, 1024], F32, tag="st", bufs=2)
                        kcols = slice(kb * 128, (kb + 1) * 128)
                        qcols = slice(j * QW + lo, (j + 1) * QW)
                        nc.tensor.matmul(
                            st[:, lo:QW],
                            kt[0:64, p, kcols],
                            qt[0:64, p, qcols],
                            start=True, stop=True, tile_position=(0, 0),
                        )
                        nc.tensor.matmul(
                            st[:, QW + lo:2 * QW],
                            kt[64:128, p, kcols],
                            qt[64:128, p, qcols],
                            start=True, stop=True, tile_position=(64, 0),
                        )
                        pt = work.tile([128, 1024], BF16, tag="pt", bufs=3)
                        nc.scalar.activation(
                            pt[:].rearrange("p (h q) -> p h q", h=2)[:, :, lo:QW],
                            st[:].rearrange("p (h q) -> p h q", h=2)[:, :, lo:QW],
                            AF.Exp,
                        )
                        if o >= 0:
                            nc.vector.tensor_mul(pt[:, lo:QW], pt[:, lo:QW], m0[:, 0:W])
                            nc.vector.tensor_mul(
                                pt[:, QW + lo:2 * QW], pt[:, QW + lo:2 * QW], m0[:, 0:W]
                            )
                        nc.tensor.matmul(
                            o_A[:, lo:QW],
                            v[:, kb, hA * 65:(hA + 1) * 65],
                            pt[:, lo:QW],
                            start=(kb == 0), stop=(kb == nkb - 1),
                        )
                        nc.tensor.matmul(
                            o_B[:, lo:QW],
                            v[:, kb, hB * 65:(hB + 1) * 65],
                            pt[:, QW + lo:2 * QW],
                            start=(kb == 0), stop=(kb == nkb - 1),
                        )
                    # normalize: ot[:, p, jrange] = o / rowsum
                    # copy psum accumulators out first so the o slots free early
                    ocp = work.tile([65, 1024], F32, tag="ocp", bufs=3)
                    nc.vector.tensor_copy(ocp[:, 0:QW], o_A[:])
                    nc.vector.tensor_copy(ocp[:, QW:1024], o_B[:])
                    rec = work.tile([1, 1024], F32, tag="rec", bufs=2)
                    nc.vector.reciprocal(rec[:, 0:QW], ocp[64:65, 0:QW])
                    nc.vector.reciprocal(rec[:, QW:1024], ocp[64:65, QW:1024])
                    bc = work.tile([64, 1024], F32, tag="bc", bufs=2)
                    nc.gpsimd.partition_broadcast(bc[:, 0:QW], rec[:, 0:QW], channels=64)
                    nc.gpsimd.partition_broadcast(bc[:, QW:1024], rec[:, QW:1024], channels=64)
                    jr = slice(j * QW, (j + 1) * QW)
                    nc.vector.tensor_mul(ot[0:64, p, jr], ocp[0:64, 0:QW], bc[:, 0:QW])
                    nc.vector.tensor_mul(ot[64:128, p, jr], ocp[0:64, QW:1024], bc[:, QW:1024])

                # ---------------- phase 3(j): output projection + RS ----------------
                jr = slice(j * QW, (j + 1) * QW)
                for n in range(8):
                    yps = psum.tile([128, QW], F32, tag="st", bufs=2)
                    for c in range(4):
                        nc.tensor.matmul(
                            yps[:],
                            wo[:, c, n * 128:(n + 1) * 128],
                            ot[:, c, jr],
                            start=(c == 0), stop=(c == 3),
                        )
                    ysb = work.tile([128, QW], F32, tag="ysb", bufs=3)
                    nc.vector.tensor_scalar_add(ysb[:], yps[:], bias[:, n:n + 1])
                    nc.sync.dma_start(
                        out=yt_part[j, n * 128:(n + 1) * 128, :], in_=ysb[:]
                    )
                nc.gpsimd.collective_compute(
                    "ReduceScatter",
                    ALU.add,
                    replica_groups=RG,
                    ins=[yt_part[j].opt()],
                    outs=[yt_rs[j].opt()],
                )
                nc.sync.dma_start(out=yt_d[:, jr], in_=yt_rs[j])

    nc.finalize()
    return nc


def _prep_inputs(x, Wq, Wk, Wv, Wo, bo):
    """Build the 8 per-core input maps (host-side layout prep only)."""
    import ml_dtypes

    scale = 1.0 / np.sqrt(np.float32(HD))
    kr = np.arange(128, dtype=np.float32)[:, None]
    qc = np.arange(512, dtype=np.float32)[None, :]
    m0 = (qc >= kr).astype(ml_dtypes.bfloat16)

    in_maps = []
    for c in range(NCORES):
        b, g = c // 2, c % 2
        hs = slice(g * 8, (g + 1) * 8)
        xt = np.ascontiguousarray(x[b].T).astype(ml_dtypes.bfloat16)
        wq = np.ascontiguousarray(Wq[hs].reshape(512, D).T * scale).astype(ml_dtypes.bfloat16)
        wk = np.ascontiguousarray(Wk[hs].reshape(512, D).T).astype(ml_dtypes.bfloat16)
        wv = np.ascontiguousarray(Wv[hs].reshape(512, D).T).astype(ml_dtypes.bfloat16)
        wo = np.ascontiguousarray(Wo[:, g * 512:(g + 1) * 512].T).astype(ml_dtypes.bfloat16)
        if g == 0:
            bias = np.ascontiguousarray(bo.reshape(8, 128).T)
        else:
            bias = np.zeros((128, 8), np.float32)
        in_maps.append(
            {"xt": xt, "wq": wq, "wk": wk, "wv": wv, "wo": wo, "bias": bias, "mask": m0}
        )
    return in_maps


def _run(inputs, trace=False, trace_cores=None):
    from concourse.bass_utils import run_bass_kernel_spmd

    if "nc" not in _CACHE:
        _CACHE["nc"] = _build_nc()
    nc = _CACHE["nc"]
    in_maps = _prep_inputs(
        inputs["x"], inputs["Wq"], inputs["Wk"], inputs["Wv"], inputs["Wo"], inputs["bo"]
    )
    r = run_bass_kernel_spmd(
        nc, in_maps, list(range(NCORES)), trace=trace, trace_cores=trace_cores
    )
    y = np.empty((B, T, D), np.float32)
    for b in range(B):
        yt = np.concatenate([r.results[2 * b]["yt"], r.results[2 * b + 1]["yt"]], axis=0)
        y[b] = yt.T.astype(np.float32)
    return y, r


def kernel(**inputs):
    y, _ = _run(inputs, trace=False)
    return y

